# revision 1
# baseline (speedup 1.0000x reference)
"""Multi-head attention (B=2, S=2048, D=1024, H=16) on 8 TRN2 NeuronCores.

Sharding: tensor-parallel over heads (2 heads/core).  Each core computes
qkv projection for its heads (full sequence), attention, then an AllToAll
redistributes the attention outputs so each core holds *all* heads for a
1/8 slice of the (batch*seq) rows and runs the output projection locally.
No cross-core reduction is needed.

Compute dtype: bf16 matmul operands, fp32 PSUM accumulation, fp32 softmax
denominators (via an appended ones-column in V so row sums fall out of the
attention matmul).
"""

import sys

sys.path.insert(0, "/opt/trn_rl_repo")

import numpy as np
import ml_dtypes

B, S, D = 2, 2048, 1024
H, HD = 16, 64
NCORES = 8
BS = B * S                 # 4096 flattened rows
HL = H // NCORES           # 2 local heads
CH = HL * HD               # 128 local q/k/v channels
ROWS = BS // NCORES        # 512 output rows per core
P = 128
NDC = D // P               # 8 chunks of the contraction dim D
NST = BS // P              # 32 seq tiles
NKB = S // P               # 16 key blocks per batch
NQC = S // 512             # 4 query chunks (512 wide) per batch
HD1 = HD + 1               # value channels + ones column

_CACHE = {}


def _build_program(with_bias: bool, local_a2a: bool = False):
    """Build (and compile) the SPMD Bass program. Same program on all cores;
    per-core behaviour differs only through input data."""
    import concourse.bass as bass
    import concourse.mybir as mybir
    import concourse.tile as tile
    from concourse import bacc
    from concourse.masks import make_identity
    from contextlib import ExitStack

    dt = mybir.dt
    AF = mybir.ActivationFunctionType
    bf, f32 = dt.bfloat16, dt.float32

    nc = bacc.Bacc()

    x = nc.dram_tensor("x", [BS, D], f32, kind="ExternalInput")
    wq = nc.dram_tensor("wq", [P, NDC, CH], bf, kind="ExternalInput")
    wk = nc.dram_tensor("wk", [P, NDC, CH], bf, kind="ExternalInput")
    wv = nc.dram_tensor("wv", [P, NDC, CH], bf, kind="ExternalInput")
    wo = nc.dram_tensor("wo", [P, NCORES, D], bf, kind="ExternalInput")
    if with_bias:
        bq = nc.dram_tensor("bq", [1, CH], bf, kind="ExternalInput")
        bk = nc.dram_tensor("bk", [1, CH], bf, kind="ExternalInput")
        bv = nc.dram_tensor("bv", [1, CH], bf, kind="ExternalInput")
        ob = nc.dram_tensor("ob", [1, D], bf, kind="ExternalInput")
    y = nc.dram_tensor("y", [ROWS, D], f32, kind="ExternalOutput")

    with tile.TileContext(nc) as tc, ExitStack() as ctx:
        const = ctx.enter_context(tc.tile_pool(name="const", bufs=1))
        ident = const.tile([P, P], bf)
        make_identity(nc, ident[:])

        wq_sb = const.tile([P, NDC, CH], bf)
        wk_sb = const.tile([P, NDC, CH], bf)
        wv_sb = const.tile([P, NDC, CH], bf)
        wo_sb = const.tile([P, NCORES, D], bf)
        nc.sync.dma_start(out=wq_sb[:], in_=wq[:])
        nc.sync.dma_start(out=wk_sb[:], in_=wk[:])
        nc.sync.dma_start(out=wv_sb[:], in_=wv[:])
        nc.sync.dma_start(out=wo_sb[:], in_=wo[:])
        if with_bias:
            bq_sb = const.tile([1, CH], bf)
            bk_sb = const.tile([1, CH], bf)
            bv_sb = const.tile([1, CH], bf)
            ob_sb = const.tile([1, D], bf)
            ones_row = const.tile([1, 512], bf)
            nc.sync.dma_start(out=bq_sb[:], in_=bq[:])
            nc.sync.dma_start(out=bk_sb[:], in_=bk[:])
            nc.sync.dma_start(out=bv_sb[:], in_=bv[:])
            nc.sync.dma_start(out=ob_sb[:], in_=ob[:])
            nc.vector.memset(ones_row[:], 1.0)

        big = ctx.enter_context(tc.tile_pool(name="big", bufs=1))
        xT = big.tile([P, NDC, BS], bf)        # x transposed: [d%128, d//128, row]
        qT = big.tile([P, BS], bf)             # q channel-major
        kT = big.tile([P, BS], bf)             # k channel-major
        v_aug = big.tile([P, NST, HL * HD1], bf)  # v row-major + ones cols
        valsT = big.tile([P, BS], bf)          # attention out, channel-major
        vfull = big.tile([P, NCORES, ROWS], bf)  # all heads, this core's rows

        # ones columns of v_aug (never overwritten by the v evictions)
        for h in range(HL):
            nc.vector.memset(v_aug[:, :, h * HD1 + HD], 1.0)

        xin = ctx.enter_context(tc.tile_pool(name="xin", bufs=3))
        expp = ctx.enter_context(tc.tile_pool(name="expp", bufs=2))
        small = ctx.enter_context(tc.tile_pool(name="small", bufs=4))
        outp = ctx.enter_context(tc.tile_pool(name="outp", bufs=3))

        # PSUM: 8 banks total. 2 (pt) + 2 (pbig) + 2 (pscore) + 2 (pav).
        pt = ctx.enter_context(tc.tile_pool(name="pt", bufs=2, space="PSUM"))
        pbig = ctx.enter_context(tc.tile_pool(name="pbig", bufs=2, space="PSUM"))
        pscore = ctx.enter_context(tc.tile_pool(name="pscore", bufs=2, space="PSUM"))
        pav = ctx.enter_context(tc.tile_pool(name="pav", bufs=2, space="PSUM"))

        # ---- phase A/B: load+cast x, transpose, v projection ----
        for st in range(NST):
            x_bf = xin.tile([P, D], bf, tag="xbf")
            nc.gpsimd.dma_start(out=x_bf[:], in_=x[st * P:(st + 1) * P, :])
            for c in range(NDC):
                ptile = pt.tile([P, P], bf, tag="ptr")
                nc.tensor.transpose(ptile[:], x_bf[:, c * P:(c + 1) * P], ident[:])
                nc.vector.tensor_copy(
                    out=xT[:, c, st * P:(st + 1) * P], in_=ptile[:]
                )
            pv = pbig.tile([P, CH], f32, tag="pk")
            for c in range(NDC):
                nc.tensor.matmul(
                    pv[:],
                    lhsT=xT[:, c, st * P:(st + 1) * P],
                    rhs=wv_sb[:, c, :],
                    start=(c == 0),
                    stop=(c == NDC - 1 and not with_bias),
                )
            if with_bias:
                nc.tensor.matmul(
                    pv[:], lhsT=ones_row[:, 0:P], rhs=bv_sb[:],
                    start=False, stop=True,
                )
            for h in range(HL):
                nc.vector.tensor_copy(
                    out=v_aug[:, st, h * HD1:h * HD1 + HD],
                    in_=pv[:, h * HD:(h + 1) * HD],
                )

        # ---- q/k projections (channel-major) ----
        for b in range(B):
            for qc in range(NQC):
                base = b * S + qc * 512
                for (wsb, bsb, dst) in (
                    (wq_sb, "bq", qT),
                    (wk_sb, "bk", kT),
                ):
                    pq = pbig.tile([P, 512], f32, tag="pk")
                    for c in range(NDC):
                        nc.tensor.matmul(
                            pq[:],
                            lhsT=wsb[:, c, :],
                            rhs=xT[:, c, base:base + 512],
                            start=(c == 0),
                            stop=(c == NDC - 1 and not with_bias),
                        )
                    if with_bias:
                        nc.tensor.matmul(
                            pq[:],
                            lhsT=(bq_sb if bsb == "bq" else bk_sb)[:],
                            rhs=ones_row[:],
                            start=False, stop=True,
                        )
                    nc.vector.tensor_copy(out=dst[:, base:base + 512], in_=pq[:])

        # ---- attention per (batch, head) ----
        for b in range(B):
            for h in range(HL):
                hp = h * HD
                for qc in range(NQC):
                    qbase = b * S + qc * 512
                    et = expp.tile([P, NKB, 512], bf, tag="exp")
                    for kb in range(NKB):
                        kbase = b * S + kb * P
                        ps = pscore.tile([P, 512], f32, tag="ps")
                        nc.tensor.matmul(
                            ps[:],
                            lhsT=kT[hp:hp + HD, kbase:kbase + P],
                            rhs=qT[hp:hp + HD, qbase:qbase + 512],
                            start=True,
                            stop=True,
                        )
                        nc.scalar.activation(
                            et[:, kb, :], ps[:], AF.Exp, scale=0.125
                        )
                    for qt in range(4):
                        pa = pav.tile([P, HD1], f32, tag="pa")
                        for kb in range(NKB):
                            nc.tensor.matmul(
                                pa[:],
                                lhsT=et[:, kb, qt * P:(qt + 1) * P],
                                rhs=v_aug[:, b * NKB + kb, h * HD1:(h + 1) * HD1],
                                start=(kb == 0),
                                stop=(kb == NKB - 1),
                            )
                        rc = small.tile([P, 1], f32, tag="rc")
                        nc.vector.reciprocal(rc[:], pa[:, HD:HD1])
                        vn = small.tile([P, HD], bf, tag="vn")
                        nc.vector.tensor_scalar_mul(vn[:], pa[:, 0:HD], rc[:])
                        ptv = pt.tile([P, P], bf, tag="ptr")
                        nc.tensor.transpose(ptv[hp:hp + HD, :], vn[:], ident[:])
                        col = qbase + qt * P
                        nc.vector.tensor_copy(
                            out=valsT[hp:hp + HD, col:col + P],
                            in_=ptv[hp:hp + HD, :],
                        )

        # ---- AllToAll: vals (2 local heads, all rows) -> (all heads, local rows)
        dram = ctx.enter_context(tc.tile_pool(name="dram", bufs=1, space="DRAM"))
        cc_in = dram.tile([NCORES, P, ROWS], bf)
        cc_out = dram.tile([NCORES, P, ROWS], bf)
        for j in range(NCORES):
            nc.sync.dma_start(
                out=cc_in[j], in_=valsT[:, j * ROWS:(j + 1) * ROWS]
            )
        if local_a2a:
            # timing-model stand-in: local copy instead of the collective
            nc.sync.dma_start(out=cc_out[:], in_=cc_in[:])
        else:
            import concourse.mybir as mybir2
            nc.gpsimd.collective_compute(
                "AllToAll",
                mybir2.AluOpType.bypass,
                replica_groups=[list(range(NCORES))],
                ins=[cc_in[:]],
                outs=[cc_out[:]],
            )
        nc.sync.dma_start(out=vfull[:], in_=cc_out.rearrange("i p r -> p i r"))

        # ---- output projection ----
        for rt in range(ROWS // P):
            for dh in range(D // 512):
                po = pbig.tile([P, 512], f32, tag="pk")
                for c in range(NCORES):
                    nc.tensor.matmul(
                        po[:],
                        lhsT=vfull[:, c, rt * P:(rt + 1) * P],
                        rhs=wo_sb[:, c, dh * 512:(dh + 1) * 512],
                        start=(c == 0),
                        stop=(c == NCORES - 1 and not with_bias),
                    )
                if with_bias:
                    nc.tensor.matmul(
                        po[:], lhsT=ones_row[:, 0:P],
                        rhs=ob_sb[:, dh * 512:(dh + 1) * 512],
                        start=False, stop=True,
                    )
                osb = outp.tile([P, 512], f32, tag="osb")
                nc.vector.tensor_copy(out=osb[:], in_=po[:])
                nc.sync.dma_start(
                    out=y[rt * P:(rt + 1) * P, dh * 512:(dh + 1) * 512],
                    in_=osb[:],
                )

    nc.compile()
    return nc


def get_program(with_bias: bool, local_a2a: bool = False):
    key = (with_bias, local_a2a)
    if key not in _CACHE:
        _CACHE[key] = _build_program(with_bias, local_a2a)
    return _CACHE[key]


def make_in_maps(x, qkv_w, qkv_b, o_w, o_b):
    """Host-side sharding: slice per-head weight rows, transpose to the
    layouts the kernel consumes, cast weights to bf16."""
    bfnp = ml_dtypes.bfloat16
    x2 = np.ascontiguousarray(np.asarray(x, np.float32).reshape(BS, D))

    qkv_w = np.asarray(qkv_w, np.float32)
    o_w = np.asarray(o_w, np.float32)
    qkv_b = np.asarray(qkv_b, np.float32)
    o_b = np.asarray(o_b, np.float32)

    with_bias = bool(np.any(qkv_b) or np.any(o_b))

    # o_w.T arranged [p, chunk, dout]: channel ch = chunk*128 + p
    woT = np.ascontiguousarray(
        o_w.T.reshape(NCORES, P, D).transpose(1, 0, 2).astype(bfnp)
    )
    ob_host = np.ascontiguousarray(o_b.reshape(1, D).astype(bfnp))

    in_maps = []
    for m in range(NCORES):
        heads = [m * HL + h for h in range(HL)]
        q_rows = np.concatenate([qkv_w[h * 3 * HD:h * 3 * HD + HD] for h in heads])
        k_rows = np.concatenate(
            [qkv_w[h * 3 * HD + HD:h * 3 * HD + 2 * HD] for h in heads]
        )
        v_rows = np.concatenate(
            [qkv_w[h * 3 * HD + 2 * HD:h * 3 * HD + 3 * HD] for h in heads]
        )

        def wt(rows):
            # [CH, D] -> [D, CH] -> [p, chunk, CH]
            return np.ascontiguousarray(
                rows.T.reshape(NDC, P, CH).transpose(1, 0, 2).astype(bfnp)
            )

        im = {
            "x": x2,
            "wq": wt(q_rows),
            "wk": wt(k_rows),
            "wv": wt(v_rows),
            "wo": woT,
        }
        if with_bias:
            bqv = np.concatenate(
                [qkv_b[h * 3 * HD:h * 3 * HD + HD] for h in heads]
            )
            bkv = np.concatenate(
                [qkv_b[h * 3 * HD + HD:h * 3 * HD + 2 * HD] for h in heads]
            )
            bvv = np.concatenate(
                [qkv_b[h * 3 * HD + 2 * HD:h * 3 * HD + 3 * HD] for h in heads]
            )
            im["bq"] = np.ascontiguousarray(bqv.reshape(1, CH).astype(bfnp))
            im["bk"] = np.ascontiguousarray(bkv.reshape(1, CH).astype(bfnp))
            im["bv"] = np.ascontiguousarray(bvv.reshape(1, CH).astype(bfnp))
            im["ob"] = ob_host
        in_maps.append(im)
    return in_maps, with_bias


def kernel(x, qkv_w, qkv_b, o_w, o_b):
    from concourse.bass_utils import run_bass_kernel_spmd

    in_maps, with_bias = make_in_maps(x, qkv_w, qkv_b, o_w, o_b)
    nc = get_program(with_bias)
    res = run_bass_kernel_spmd(nc, in_maps, list(range(NCORES)))
    out = np.concatenate([res.results[m]["y"] for m in range(NCORES)], axis=0)
    return np.ascontiguousarray(out.reshape(B, S, D))


# revision 20
# speedup vs baseline: 297.6966x; 297.6966x over previous
"""Multi-head attention (B=2, S=2048, D=1024, H=16) on 8 TRN2 NeuronCores.

Sharding: tensor-parallel over heads (2 heads/core).  Each core computes
the qkv projection for its heads (full sequence) and attention, then an
AllToAll redistributes attention outputs so each core holds *all* heads
for a 1/8 slice of the (batch*seq) rows and runs the output projection
locally.  No cross-core reduction needed.

Compute dtype: bf16 matmul operands, fp32 PSUM accumulation.  Softmax
denominators come for free from a ones-column appended to V (scores are
small here, so exp without max-subtraction is safe); normalization is a
per-partition scalar multiply fused into the PSUM eviction.

Engines execute their instruction streams in order, so the emission
order below is a hand-software-pipelined schedule: scores/exp of block
i+1 are interleaved with the attention-value matmuls of block i and
with the x-transpose/projection prep of the next batch.
"""

import sys

sys.path.insert(0, "/opt/trn_rl_repo")

import numpy as np
import ml_dtypes

B, S, D = 2, 2048, 1024
H, HD = 16, 64
NCORES = 8
BS = B * S                 # 4096 flattened rows
HL = H // NCORES           # 2 local heads
CH = HL * HD               # 128 local q/k/v channels
ROWS = BS // NCORES        # 512 output rows per core
P = 128
NDC = D // P               # 8 chunks of the contraction dim D
NST = S // P               # 16 seq tiles per batch
NKB = S // P               # 16 key blocks per batch
QCW = 1024                 # query-chunk width (one exp instruction per kb)
NQC = S // QCW             # 2 query chunks per batch
HD1 = HD + 1               # value channels + ones column

_CACHE = {}

XPOSE_MODE = "pe"          # "pe" | "dma"


def _interleave(primary, secondary, lead=0):
    """Emit primary tasks in order, spreading secondary tasks between them.
    The first `lead` primary tasks are emitted before any secondary."""
    ns = len(secondary)
    npr = max(len(primary) - lead, 1)
    si = 0
    for i, p in enumerate(primary):
        p()
        tgt = (i + 1 - lead) * ns // npr if i >= lead else 0
        while si < tgt:
            secondary[si]()
            si += 1
    while si < ns:
        secondary[si]()
        si += 1


def _build_program(with_bias: bool, local_a2a: bool = False, xpose: str | None = None,
                   repeats: int = 1, loop_n: int = 0, dve_cast: bool = False):
    import concourse.bass as bass
    import concourse.mybir as mybir
    import concourse.tile as tile
    from concourse import bacc
    from concourse.masks import make_identity
    from contextlib import ExitStack

    xpose = xpose or XPOSE_MODE
    dt = mybir.dt
    AF = mybir.ActivationFunctionType
    bf, f32 = dt.bfloat16, dt.float32

    nc = bacc.Bacc()

    x = nc.dram_tensor("x", [BS, D], f32, kind="ExternalInput")
    wq = nc.dram_tensor("wq", [P, NDC, CH], bf, kind="ExternalInput")
    wk = nc.dram_tensor("wk", [P, NDC, CH], bf, kind="ExternalInput")
    wv = nc.dram_tensor("wv", [P, NDC, CH], bf, kind="ExternalInput")
    wo = nc.dram_tensor("wo", [P, NCORES, D], bf, kind="ExternalInput")
    if with_bias:
        bq = nc.dram_tensor("bq", [1, CH], bf, kind="ExternalInput")
        bk = nc.dram_tensor("bk", [1, CH], bf, kind="ExternalInput")
        bv = nc.dram_tensor("bv", [1, CH], bf, kind="ExternalInput")
        ob = nc.dram_tensor("ob", [1, D], bf, kind="ExternalInput")
    y = nc.dram_tensor("y", [ROWS, D], f32, kind="ExternalOutput")

    # weight loads: HWDGE in pe mode; SWDGE in dma mode so the xbar
    # transposes don't interleave with copy-mode HWDGE transfers
    wload = (lambda **kw: nc.sync.dma_start(**kw)) if xpose == "pe" else (
        lambda **kw: nc.gpsimd.dma_start(**kw))

    with tile.TileContext(nc) as tc, ExitStack() as ctx:
        const = ctx.enter_context(tc.tile_pool(name="const", bufs=1))
        ident = const.tile([P, P], bf)
        make_identity(nc, ident[:])

        wq_sb = const.tile([P, NDC, CH], bf)
        wk_sb = const.tile([P, NDC, CH], bf)
        wv_sb = const.tile([P, NDC, CH], bf)
        wo_sb = const.tile([P, NCORES, D], bf)
        if with_bias:
            bq_sb = const.tile([1, CH], bf)
            bk_sb = const.tile([1, CH], bf)
            bv_sb = const.tile([1, CH], bf)
            ob_sb = const.tile([1, D], bf)
            ones_row = const.tile([1, 512], bf)

        big = ctx.enter_context(tc.tile_pool(name="big", bufs=1))
        # xT and vfull share one slot: vfull is written only after the last
        # read of xT (the q/k/v projections), so the WAR dep is harmless.
        xT = big.tile([P, NDC, BS], bf, tag="bigbuf")       # [d%128, d//128, row]
        qT = big.tile([P, BS], bf)                          # q channel-major
        kT = big.tile([P, BS], bf)                          # k channel-major
        v_aug = big.tile([P, B * NST, HL * HD1], bf)        # v row-major + ones
        valsT = big.tile([P, BS], bf)                       # attn out, ch-major

        xin = ctx.enter_context(tc.tile_pool(name="xin", bufs=3))
        expp = ctx.enter_context(tc.tile_pool(name="expp", bufs=2))
        small = ctx.enter_context(tc.tile_pool(name="small", bufs=4))
        outp = ctx.enter_context(tc.tile_pool(name="outp", bufs=2))

        # PSUM budget: 8 banks = pscore 2x[128,1024](4) + pt 2 + pbig 1 + pav 1
        pt = ctx.enter_context(tc.tile_pool(name="pt", bufs=2, space="PSUM"))
        pbig = ctx.enter_context(tc.tile_pool(name="pbig", bufs=1, space="PSUM"))
        pscore = ctx.enter_context(tc.tile_pool(name="pscore", bufs=2, space="PSUM"))
        pav = ctx.enter_context(tc.tile_pool(name="pav", bufs=1, space="PSUM"))

        dram = ctx.enter_context(tc.tile_pool(name="dram", bufs=1, space="DRAM"))
        # the AllToAll is split into two half-payload collectives (head 0 /
        # head 1 channel halves) so the first can run under live attention
        ccA_in = dram.tile([NCORES, HD, ROWS], bf)
        ccA_out = dram.tile([NCORES, HD, ROWS], bf)
        ccB_in = dram.tile([NCORES, HD, ROWS], bf)
        ccB_out = dram.tile([NCORES, HD, ROWS], bf)
        if xpose == "dma":
            xbf_dram = dram.tile([BS, D], bf)

        # ones columns for the softmax-denominator trick; value columns are
        # overwritten by the v-projection evictions
        for h in range(HL):
            nc.vector.memset(v_aug[:, :, h * HD1 + HD], 1.0)

        # ---------------- task builders ----------------

        def t_wload(wsb, wdram):
            return lambda: wload(out=wsb[:], in_=wdram[:])

        def t_bias_loads():
            def go():
                wload(out=bq_sb[:], in_=bq[:])
                wload(out=bk_sb[:], in_=bk[:])
                wload(out=bv_sb[:], in_=bv[:])
                wload(out=ob_sb[:], in_=ob[:])
                nc.vector.memset(ones_row[:], 1.0)
            return go

        def t_xpose_pe(st):
            def go():
                x_bf = xin.tile([P, D], bf, tag="xbf", name="x_bf")
                if dve_cast:
                    x_f = xin.tile([P, D], f32, tag="xf", name="x_f", bufs=2)
                    nc.sync.dma_start(out=x_f[:], in_=x[st * P:(st + 1) * P, :])
                    nc.vector.tensor_copy(out=x_bf[:], in_=x_f[:])
                else:
                    nc.gpsimd.dma_start(out=x_bf[:], in_=x[st * P:(st + 1) * P, :])
                for c in range(NDC):
                    ptile = pt.tile([P, P], bf, tag="ptr", name="ptile")
                    nc.tensor.transpose(
                        ptile[:], x_bf[:, c * P:(c + 1) * P], ident[:]
                    )
                    nc.vector.tensor_copy(
                        out=xT[:, c, st * P:(st + 1) * P], in_=ptile[:]
                    )
            return go

        def t_xcast_dma(b, rc):
            def go():
                r0 = b * S + rc * 512
                nc.gpsimd.dma_start(
                    out=xbf_dram[r0:r0 + 512, :], in_=x[r0:r0 + 512, :]
                )
            return go

        def t_xpose_dma(b, rc, c):
            def go():
                r0 = b * S + rc * 512
                nc.sync.dma_start(
                    out=xT[:, c, r0:r0 + 512],
                    in_=xbf_dram[r0:r0 + 512, c * P:(c + 1) * P],
                    transpose=True,
                )
            return go

        def t_vproj(st):
            def go():
                pv = pbig.tile([P, CH], f32, tag="pk", name="pv")
                for c in range(NDC):
                    nc.tensor.matmul(
                        pv[:],
                        lhsT=xT[:, c, st * P:(st + 1) * P],
                        rhs=wv_sb[:, c, :],
                        start=(c == 0),
                        stop=(c == NDC - 1 and not with_bias),
                    )
                if with_bias:
                    nc.tensor.matmul(
                        pv[:], lhsT=ones_row[:, 0:P], rhs=bv_sb[:],
                        start=False, stop=True,
                    )
                for h in range(HL):
                    nc.vector.tensor_copy(
                        out=v_aug[:, st, h * HD1:h * HD1 + HD],
                        in_=pv[:, h * HD:(h + 1) * HD],
                    )
            return go

        def t_kqproj(b, which, qc):
            def go():
                wsb, dst = (wk_sb, kT) if which == "k" else (wq_sb, qT)
                base = b * S + qc * 512
                pq = pbig.tile([P, 512], f32, tag="pk", name="pq")
                for c in range(NDC):
                    nc.tensor.matmul(
                        pq[:],
                        lhsT=wsb[:, c, :],
                        rhs=xT[:, c, base:base + 512],
                        start=(c == 0),
                        stop=(c == NDC - 1 and not with_bias),
                    )
                if with_bias:
                    nc.tensor.matmul(
                        pq[:],
                        lhsT=(bk_sb if which == "k" else bq_sb)[:],
                        rhs=ones_row[:],
                        start=False, stop=True,
                    )
                nc.vector.tensor_copy(out=dst[:, base:base + 512], in_=pq[:])
            return go

        def prep_A_tasks(b):
            """x load/cast/transpose + v projection; one task pair per seq
            tile (returned flat, in order)."""
            tasks = []
            if xpose == "pe":
                for t in range(NST):
                    tasks.append(t_xpose_pe(b * NST + t))
                    tasks.append(t_vproj(b * NST + t))
            else:
                for rc in range(4):
                    tasks.append(t_xcast_dma(b, rc))
                    for c in range(NDC):
                        tasks.append(t_xpose_dma(b, rc, c))
                    for tt in range(4):
                        tasks.append(t_vproj(b * NST + rc * 4 + tt))
            return tasks

        # attention blocks: per (b, h, qc) -> score tasks (one per kb) and
        # AV tasks (one per qt)
        def score_tasks(b, h, qc, et):
            hp = h * HD
            qbase = b * S + qc * QCW
            tasks = []

            def mk(kb):
                def go():
                    kbase = b * S + kb * P
                    ps = pscore.tile([P, QCW], f32, tag="ps", name="ps")
                    for qh in range(QCW // 512):
                        nc.tensor.matmul(
                            ps[:, qh * 512:(qh + 1) * 512],
                            lhsT=kT[hp:hp + HD, kbase:kbase + P],
                            rhs=qT[hp:hp + HD,
                                   qbase + qh * 512:qbase + (qh + 1) * 512],
                            start=True,
                            stop=True,
                        )
                    nc.scalar.activation(et[:, kb, :], ps[:], AF.Exp, scale=0.125)
                return go

            for kb in range(NKB):
                tasks.append(mk(kb))
            return tasks

        def av_tasks(b, h, qc, et):
            hp = h * HD
            qbase = b * S + qc * QCW
            tasks = []

            def mk(qt):
                def go():
                    pa = pav.tile([P, HD1], f32, tag="pa", name="pa")
                    for kb in range(NKB):
                        nc.tensor.matmul(
                            pa[:],
                            lhsT=et[:, kb, qt * P:(qt + 1) * P],
                            rhs=v_aug[:, b * NKB + kb, h * HD1:(h + 1) * HD1],
                            start=(kb == 0),
                            stop=(kb == NKB - 1),
                        )
                    rc_ = small.tile([P, 1], f32, tag="rc", name="rc")
                    nc.vector.reciprocal(rc_[:], pa[:, HD:HD1])
                    vn = small.tile([P, HD], bf, tag="vn", name="vn")
                    nc.vector.tensor_scalar_mul(vn[:], pa[:, 0:HD], rc_[:])
                    ptv = pt.tile([P, P], bf, tag="ptr", name="ptv")
                    nc.tensor.transpose(ptv[hp:hp + HD, :], vn[:], ident[:])
                    col = qbase + qt * P
                    nc.vector.tensor_copy(
                        out=valsT[hp:hp + HD, col:col + P],
                        in_=ptv[hp:hp + HD, :],
                    )
                return go

            for qt in range(QCW // P):
                tasks.append(mk(qt))
            return tasks

        def t_ccdma(half, j):
            ccin = ccA_in if half == 0 else ccB_in
            hp = half * HD
            return lambda: nc.sync.dma_start(
                out=ccin[j], in_=valsT[hp:hp + HD, j * ROWS:(j + 1) * ROWS]
            )

        def t_a2a(half):
            ccin, ccout = (ccA_in, ccA_out) if half == 0 else (ccB_in, ccB_out)

            def go():
                if local_a2a:
                    nc.sync.dma_start(out=ccout[:], in_=ccin[:])
                else:
                    nc.gpsimd.collective_compute(
                        "AllToAll",
                        mybir.AluOpType.bypass,
                        replica_groups=[list(range(NCORES))],
                        ins=[ccin[:]],
                        outs=[ccout[:]],
                    )
            return go

        # ---------------- emission (software pipeline) ----------------
        def emit_body(load_weights):
            if load_weights:
                t_wload(wv_sb, wv)()
                t_wload(wk_sb, wk)()
                t_wload(wq_sb, wq)()
                if with_bias:
                    t_bias_loads()()

            A0 = prep_A_tasks(0)        # per seq tile: [xpose, vproj] pairs
            A1 = prep_A_tasks(1)
            # front: enough of batch 0 to start scoring, k/q chunks woven in
            if xpose == "pe":
                for task in A0[0:8]:    # seq tiles 0..3
                    task()
                t_kqproj(0, "k", 0)()
                for task in A0[8:16]:   # seq tiles 4..7
                    task()
            else:
                for task in A0:
                    task()
                t_kqproj(0, "k", 0)()
            t_kqproj(0, "q", 0)()
            t_kqproj(0, "q", 1)()

            # h-major block order per batch: the head-0 half of valsT is
            # complete after block 5, letting the first half-AllToAll run
            # under the remaining head-1 attention.
            block_ids = [(b, h, qc) for b in range(B) for h in range(HL)
                         for qc in range(NQC)]
            qp_rest = {b: [t_kqproj(b, "q", c) for c in (2, 3)]
                       for b in range(B)}
            warm1 = [t_kqproj(1, "k", 0), t_kqproj(1, "q", 0),
                     t_kqproj(1, "q", 1)]
            vfull = big.tile([P, NCORES, ROWS], bf, name="vfull")

            def t_vfull(half):
                ccout = ccA_out if half == 0 else ccB_out
                hp = half * HD
                return lambda: nc.sync.dma_start(
                    out=vfull[hp:hp + HD, :, :],
                    in_=ccout.rearrange("i p r -> p i r"),
                )

            # extra tasks joining the mix at a given global block index
            # (cc DMAs depend on AV tasks which lag their block by one)
            extras = {
                2: [t_ccdma(0, 0), t_ccdma(0, 1)],
                3: ([t_wload(wo_sb, wo)] if load_weights else [])
                   + [t_ccdma(0, 2), t_ccdma(0, 3)],
                4: [t_ccdma(1, 0), t_ccdma(1, 1)],
                5: [t_ccdma(1, 2), t_ccdma(1, 3)],
                6: [t_ccdma(0, 4), t_ccdma(0, 5)],
                7: [t_ccdma(0, 6), t_ccdma(0, 7), t_a2a(0), t_vfull(0)],
            }
            prev_av = []
            for i, (b, h, qc) in enumerate(block_ids):
                et = expp.tile([P, NKB, QCW], bf, tag="exp", name="et")
                s = score_tasks(b, h, qc, et)
                if h == 0 and qc == 0:
                    kp = [t_kqproj(b, "k", c) for c in (1, 2, 3)]
                    qp = qp_rest[b]
                    if b == 0 and xpose == "pe":
                        # explicit weave: remaining A tiles + k/q chunks after
                        # the A tiles they contract over
                        # (scores kb 4c..4c+3 need k chunk c <- A tiles 4c..4c+3)
                        primary = (s[0:2] + A0[16:20] + s[2:4] + kp[0:1]
                                   + A0[20:24] + s[4:6] + A0[24:28] + s[6:8]
                                   + kp[1:2] + A0[28:32] + s[8:10] + qp[0:1]
                                   + s[10:12] + kp[2:3] + qp[1:2] + s[12:16])
                    else:
                        primary = (s[0:4] + kp[0:1] + s[4:8] + kp[1:2]
                                   + s[8:12] + kp[2:3] + qp[0:1] + qp[1:2]
                                   + s[12:16])
                else:
                    primary = s
                mix = list(prev_av)
                if b == 0 and 1 <= i <= 2:
                    lo = (i - 1) * len(A1) // 2
                    hi = i * len(A1) // 2
                    mix += A1[lo:hi]
                if i == 3:
                    mix = warm1 + mix
                mix += extras.get(i, [])
                _interleave(primary, mix, lead=2)
                prev_av = av_tasks(b, h, qc, et)
            for task in prev_av:
                task()
            t_ccdma(1, 4)()
            t_ccdma(1, 5)()
            t_ccdma(1, 6)()
            t_ccdma(1, 7)()
            t_a2a(1)()
            t_vfull(1)()

            # ---- output projection ----
            for rt in range(ROWS // P):
                for dh in range(D // 512):
                    po = pscore.tile([P, 512], f32, tag="ps", name="po")
                    for c in range(NCORES):
                        nc.tensor.matmul(
                            po[:],
                            lhsT=vfull[:, c, rt * P:(rt + 1) * P],
                            rhs=wo_sb[:, c, dh * 512:(dh + 1) * 512],
                            start=(c == 0),
                            stop=(c == NCORES - 1 and not with_bias),
                        )
                    if with_bias:
                        nc.tensor.matmul(
                            po[:], lhsT=ones_row[:, 0:P],
                            rhs=ob_sb[:, dh * 512:(dh + 1) * 512],
                            start=False, stop=True,
                        )
                    osb = outp.tile([P, 512], f32, tag="osb", name="osb")
                    nc.vector.tensor_copy(out=osb[:], in_=po[:])
                    nc.sync.dma_start(
                        out=y[rt * P:(rt + 1) * P, dh * 512:(dh + 1) * 512],
                        in_=osb[:],
                    )

        if loop_n > 1:
            t_wload(wv_sb, wv)()
            t_wload(wk_sb, wk)()
            t_wload(wq_sb, wq)()
            t_wload(wo_sb, wo)()
            if with_bias:
                t_bias_loads()()
            with tc.For_i(0, loop_n, 1):
                emit_body(load_weights=False)
        else:
            for rep in range(repeats):
                emit_body(load_weights=(rep == 0))

    nc.compile()
    return nc


def get_program(with_bias: bool, local_a2a: bool = False, xpose: str | None = None,
                repeats: int = 1, loop_n: int = 0, dve_cast: bool = False):
    key = (with_bias, local_a2a, xpose or XPOSE_MODE, repeats, loop_n, dve_cast)
    if key not in _CACHE:
        _CACHE[key] = _build_program(with_bias, local_a2a, xpose, repeats, loop_n, dve_cast)
    return _CACHE[key]


def make_in_maps(x, qkv_w, qkv_b, o_w, o_b):
    """Host-side sharding: slice per-head weight rows, transpose to the
    layouts the kernel consumes, cast weights to bf16."""
    bfnp = ml_dtypes.bfloat16
    x2 = np.ascontiguousarray(np.asarray(x, np.float32).reshape(BS, D))

    qkv_w = np.asarray(qkv_w, np.float32)
    o_w = np.asarray(o_w, np.float32)
    qkv_b = np.asarray(qkv_b, np.float32)
    o_b = np.asarray(o_b, np.float32)

    with_bias = bool(np.any(qkv_b) or np.any(o_b))

    woT = np.ascontiguousarray(
        o_w.T.reshape(NCORES, P, D).transpose(1, 0, 2).astype(bfnp)
    )
    ob_host = np.ascontiguousarray(o_b.reshape(1, D).astype(bfnp))

    in_maps = []
    for m in range(NCORES):
        heads = [m * HL + h for h in range(HL)]
        q_rows = np.concatenate([qkv_w[h * 3 * HD:h * 3 * HD + HD] for h in heads])
        k_rows = np.concatenate(
            [qkv_w[h * 3 * HD + HD:h * 3 * HD + 2 * HD] for h in heads]
        )
        v_rows = np.concatenate(
            [qkv_w[h * 3 * HD + 2 * HD:h * 3 * HD + 3 * HD] for h in heads]
        )

        def wt(rows):
            # [CH, D] -> [D, CH] -> [p, chunk, CH]
            return np.ascontiguousarray(
                rows.T.reshape(NDC, P, CH).transpose(1, 0, 2).astype(bfnp)
            )

        im = {
            "x": x2,
            "wq": wt(q_rows),
            "wk": wt(k_rows),
            "wv": wt(v_rows),
            "wo": woT,
        }
        if with_bias:
            bqv = np.concatenate(
                [qkv_b[h * 3 * HD:h * 3 * HD + HD] for h in heads]
            )
            bkv = np.concatenate(
                [qkv_b[h * 3 * HD + HD:h * 3 * HD + 2 * HD] for h in heads]
            )
            bvv = np.concatenate(
                [qkv_b[h * 3 * HD + 2 * HD:h * 3 * HD + 3 * HD] for h in heads]
            )
            im["bq"] = np.ascontiguousarray(bqv.reshape(1, CH).astype(bfnp))
            im["bk"] = np.ascontiguousarray(bkv.reshape(1, CH).astype(bfnp))
            im["bv"] = np.ascontiguousarray(bvv.reshape(1, CH).astype(bfnp))
            im["ob"] = ob_host
        in_maps.append(im)
    return in_maps, with_bias


def kernel(x, qkv_w, qkv_b, o_w, o_b):
    from concourse.bass_utils import run_bass_kernel_spmd

    in_maps, with_bias = make_in_maps(x, qkv_w, qkv_b, o_w, o_b)
    nc = get_program(with_bias)
    res = run_bass_kernel_spmd(nc, in_maps, list(range(NCORES)))
    out = np.concatenate([res.results[m]["y"] for m in range(NCORES)], axis=0)
    return np.ascontiguousarray(out.reshape(B, S, D))
